# revision 35
# baseline (speedup 1.0000x reference)
"""Trainium2 Bass kernel for nn_MemorySystem (cosine-sim attention memory read).

reference:
    x_norm = ||x||_row (B,1); m_norm = ||m||_row (S,1)
    sims = (x @ m^T) / max(x_norm * m_norm^T, 1e-8)
    attn = softmax(8.0 * sims, axis=1)
    out  = attn @ m                       # (B, D)

Sharding: memory_bank rows split across 8 NeuronCores (8192 rows each).
Each core computes, for its shard, the un-normalized softmax numerator
O_c = exp(S_c) @ m_c (B, D) and denominator Z_c = sum_s exp (B,), using the
bounded-score property (|8*cos| <= 8) to skip the max-subtraction entirely.
Per 512-query pass, a bf16 ReduceScatter(add) over the [512, 513] partials
(O|Z) leaves each core with a fully-reduced 64-query slice; it divides O/Z
on-chip and emits those rows. The host reassembles the slices. Pass h's
finalize chain (Z colsum, staging, RS) is emitted inside pass h+1's pipeline
so its PE ops can't head-of-line-block the next pass's matmul stream; only
the last pass's ReduceScatter is exposed at the tail.

On-chip structure (per core):
  - x is normalized on load (1/||x_q|| folded in) and transposed via TensorE
    to xhatT [d, q] (bf16).
  - m shard is SBUF-resident in TWO layouts: transposed [d, s] bf16 (lhsT of
    the 1st matmul) and natural [s, d] fp8-e4m3 (rhs of the 2nd matmul).
  - scores are computed transposed, [s-tile, q], so 8/||m_s|| is a
    per-partition scalar fused into the ACT Exp; exp writes fp8-e4m3 directly
    (values in ~[0.1, 11] are safely inside e4m3 range), pair-packed so two
    adjacent s-tiles feed ONE DoubleRow fp8 matmul (contraction 256) in the
    2nd matmul -- ~1.8x fewer PE cycles there vs bf16. Z is accumulated from
    the SAME quantized fp8 pt, so the softmax renormalization cancels the
    common quantization bias (measured end-to-end rel err 7.4e-3 in numpy).
  - row norms: ACT Square (in every ACT table set -> no table thrash with
    Exp) with accum_out and scale=1/8 giving n2/64; rsqrt via DVE
    bit-trick + 2 Newton steps (no ACT Sqrt -> no table swaps).
  - m load chain: one DVE f32->bf16 cast feeds ACT Square, cheap bf16 PE
    transposes, the fp8 m_nat recast and a 16-bit PSUM copyback.
  - Z is accumulated on DVE in bf16 (pair-wide adds of the fp8 exp tiles),
    cross-partition-reduced by one ones-matmul per pass; the division
    happens after the ReduceScatter.
  - pass 0 of the query loop is emission-interleaved with the m-load loop
    so the in-order PE stream overlaps DMA/cast/norm work with matmuls.

Measured on 8 axon trn2 cores: ~297-311 us HW exec (baseline 355 us),
rel err 8.9e-3 (gate 2e-2).
"""

import sys

sys.path.insert(0, "/opt/trn_rl_repo")

import numpy as np
from contextlib import ExitStack

B, S, D = 1024, 65536, 512
NCORES = 8
S_SHARD = S // NCORES  # 8192
P = 128

ST = S_SHARD // P  # 64 s-tiles per core
QT = B // P  # 8 q-tiles
DC = D // P  # 4 d-chunks
# query passes (offset, size): each pass's ReduceScatter overlaps the next
# pass's compute; only the last one is exposed.
PASSES = [(0, 512), (512, 512)]
LAG = 5  # load runs this many s-tiles ahead of chunk-0 compute

MAGIC = 0x5F3759DF

_CACHE = {}


def _build(loop_iters=None):
    """Build the kernel. loop_iters wraps the whole body in a device-side
    repeat loop (used only for wall-clock delta timing in bench.py)."""
    import concourse.bass as bass
    import concourse.tile as tile
    from concourse import bacc, mybir
    from concourse.masks import make_identity

    f32 = mybir.dt.float32
    bf16 = mybir.dt.bfloat16
    fp8 = mybir.dt.float8e4
    u32 = mybir.dt.uint32
    AF = mybir.ActivationFunctionType
    ALU = mybir.AluOpType

    nc = bacc.Bacc(None, num_devices=NCORES)
    x_ext = nc.declare_dram_parameter("x", [B, D], f32, isOutput=False)
    m_ext = nc.declare_dram_parameter("mem", [S_SHARD, D], f32, isOutput=False)
    out_ext = nc.declare_dram_parameter("out", [B // NCORES, D], f32, isOutput=True)

    with tile.TileContext(nc) as tc, ExitStack() as ctx:
        persist = ctx.enter_context(tc.tile_pool(name="persist", bufs=1))
        loadp = ctx.enter_context(tc.tile_pool(name="load", bufs=6))
        xp = ctx.enter_context(tc.tile_pool(name="xp", bufs=QT))
        sqp = ctx.enter_context(tc.tile_pool(name="sqp", bufs=2))
        work = ctx.enter_context(tc.tile_pool(name="work", bufs=3))
        zp = ctx.enter_context(tc.tile_pool(name="zp", bufs=2))
        stp = ctx.enter_context(tc.tile_pool(name="stp", bufs=2))
        finp = ctx.enter_context(tc.tile_pool(name="finp", bufs=1))
        dram = ctx.enter_context(tc.tile_pool(name="dram", bufs=4, space="DRAM"))
        # PSUM: 8 banks total. sc(2) + o2(4) + tp(2, shared with zt) = 8
        psum_sc = ctx.enter_context(tc.tile_pool(name="psc", bufs=2, space="PSUM"))
        psum_o = ctx.enter_context(tc.tile_pool(name="po", bufs=4, space="PSUM"))
        psum_tp = ctx.enter_context(tc.tile_pool(name="ptp", bufs=2, space="PSUM"))
        psum_zt = psum_tp

        # ---- constants ----
        ident_bf = persist.tile([P, P], bf16)
        make_identity(nc, ident_bf[:])
        ident_f32 = persist.tile([P, P], f32)
        nc.vector.tensor_copy(out=ident_f32[:], in_=ident_bf[:])
        ones_f32 = persist.tile([P, 1], f32)
        nc.vector.memset(ones_f32[:], 1.0)
        ones_bf = persist.tile([P, 1], bf16)
        nc.vector.memset(ones_bf[:], 1.0)
        one_f32 = persist.tile([1, 1], f32)
        nc.vector.memset(one_f32[:], 1.0)
        magic_u = persist.tile([P, 1], u32)
        nc.vector.memset(magic_u[:], MAGIC)

        loop_cm = tc.For_i(0, loop_iters, 1) if loop_iters else None
        if loop_cm is not None:
            loop_cm.__enter__()

        # ---- persistent SBUF tensors ----
        m_nat = persist.tile([P, ST, D], fp8)  # [s%128, s//128, d] (mm2 rhs)
        mT = persist.tile([P, DC, S_SHARD], bf16)  # [d%128, d//128, s]
        xhatT = persist.tile([P, DC, B], bf16)  # [d%128, d//128, q]
        n2m = persist.tile([P, ST], f32)  # ||m_s||^2 / 64
        rs_m = persist.tile([P, ST], f32)  # 8 / ||m_s||
        rs_u = persist.tile([P, ST], u32)  # newton scratch (bit-trick y)
        rs_t = persist.tile([P, ST], f32)  # newton scratch t1
        xn2 = persist.tile([P, QT], f32)
        rs_x = persist.tile([P, QT], f32)
        xr_u = persist.tile([P, QT], u32)
        xr_t = persist.tile([P, QT], f32)

        def rsqrt_newton(dst, a, uscr, tscr, n):
            """dst = 1/sqrt(a); all APs [P, n] f32 (uscr u32)."""
            mg = magic_u[:, 0:1]
            if n > 1:
                mg = mg.to_broadcast((P, n))
            nc.vector.tensor_scalar(
                uscr, a.bitcast(u32), 1, None, ALU.logical_shift_right
            )
            nc.vector.tensor_tensor(uscr, mg, uscr, ALU.subtract)
            y = uscr.bitcast(f32)
            for it in range(2):
                out_y = dst if it == 1 else y
                nc.vector.tensor_tensor(tscr, y, y, ALU.mult)
                nc.vector.tensor_tensor(tscr, tscr, a, ALU.mult)
                nc.vector.tensor_scalar(tscr, tscr, -0.5, 1.5, ALU.mult, ALU.add)
                nc.vector.tensor_tensor(out_y, y, tscr, ALU.mult)

        # ---- per-s-tile load step (DMA, cast, norms, transpose) ----
        # One DVE cast to bf16 feeds everything downstream: ACT Square for
        # norms, cheap bf16 PE transposes, a 16-bit fp8 recast for m_nat,
        # and a 16-bit PSUM copyback -- minimizes both PE transpose cycles
        # and total DVE bytes.
        def load_tile(t):
            mf = loadp.tile([P, D], f32, tag="mf32", name=f"mf_{t}")
            nc.sync.dma_start(out=mf[:], in_=m_ext[t * P : (t + 1) * P, :])
            mb = work.tile([P, D], bf16, tag="mb16", name=f"mb_{t}")
            nc.vector.tensor_copy(out=mb[:], in_=mf[:])
            msq = sqp.tile([P, D], f32, tag="sq", name=f"msq_{t}")
            # scale=1/8: accum collects sum((m/8)^2) = n2/64; rsqrt -> 8/||m||
            nc.scalar.activation(
                out=msq[:], in_=mb[:], func=AF.Square, scale=0.125,
                accum_out=n2m[:, t : t + 1],
            )
            nc.vector.tensor_copy(out=m_nat[:, t, :], in_=mb[:])
            mtp = psum_tp.tile([P, DC * P], bf16, tag="tp", name=f"mtp_{t}")
            for c in range(DC):
                nc.tensor.transpose(
                    mtp[:, c * P : (c + 1) * P],
                    mb[:, c * P : (c + 1) * P],
                    ident_bf[:],
                )
            nc.vector.tensor_copy(
                out=mT[:, :, t * P : (t + 1) * P],
                in_=mtp[:].rearrange("p (c q) -> p c q", c=DC),
            )
            # rsqrt group AFTER the casts/copyback: keeps the tiny newton ops
            # off the DVE path that gates the PE transposes; exp only needs
            # rs_m[t] LAG-1 tiles later.
            if t % 4 == 3:
                s = slice(t - 3, t + 1)
                rsqrt_newton(rs_m[:, s], n2m[:, s], rs_u[:, s], rs_t[:, s], 4)

        loaded = set()

        def load_tile_once(t):
            if t < ST and t not in loaded:
                loaded.add(t)
                load_tile(t)

        # HAM warm-up: ~5us of dummy matmuls while the PE would otherwise
        # idle waiting for the first DMAs -- un-throttles the PE clock gate
        # (4/8 -> 8/8) before the real transpose/matmul stream begins.
        warm = psum_sc.tile([P, PASSES[0][1]], f32, tag="sc", name="warm")
        for w in range(48):
            nc.tensor.matmul(
                warm[:, 0:P], ident_bf[:], ident_bf[:], start=True, stop=True
            )

        # prime the m pipeline before x-prep so DMA/ACT start immediately
        def prime():
            for u in range(LAG):
                load_tile_once(u)

        prime()

        # ---- x prep (single DMA; xf tiles stay resident for pass 2) ----
        # rsqrt in two half-groups so the first 4 q-tiles' normalize/transpose
        # can start while the last squares are still running.
        xfs = []
        for j in range(QT):
            xf = xp.tile([P, D], f32, tag="xf", name=f"xf_{j}")
            xfs.append(xf)
            nc.sync.dma_start(out=xf[:], in_=x_ext[j * P : (j + 1) * P, :])
            xsq = sqp.tile([P, D], f32, tag="sq", name=f"xsq_{j}")
            nc.scalar.activation(
                out=xsq[:], in_=xf[:], func=AF.Square,
                accum_out=xn2[:, j : j + 1],
            )
            if j % 4 == 3:
                s = slice(j - 3, j + 1)
                rsqrt_newton(rs_x[:, s], xn2[:, s], xr_u[:, s], xr_t[:, s], 4)
        for j in range(QT):
            xhat = work.tile([P, D], bf16, tag="xhat", name=f"xhat_{j}")
            nc.vector.tensor_scalar_mul(xhat[:], xfs[j][:], rs_x[:, j : j + 1])
            xtp = psum_tp.tile([P, DC * P], bf16, tag="tp", name=f"xtp_{j}")
            for c in range(DC):
                nc.tensor.transpose(
                    xtp[:, c * P : (c + 1) * P],
                    xhat[:, c * P : (c + 1) * P],
                    ident_bf[:],
                )
            nc.vector.tensor_copy(
                out=xhatT[:, :, j * P : (j + 1) * P],
                in_=xtp[:].rearrange("p (c q) -> p c q", c=DC),
            )

        # per-chunk ReduceScatter + divide + output (bf16 on the wire: the
        # CCE adds in fp32, payload quantization adds <1e-4 to rel err)
        def _emit_rs(h, partial):
            qoff, qp = PASSES[h]
            qr = qp // NCORES
            roff = qoff // NCORES
            rsout = dram.tile([qr, D + 1], bf16, tag="rsout", name=f"rsout_{h}")
            nc.gpsimd.collective_compute(
                "ReduceScatter",
                mybir.AluOpType.add,
                replica_groups=[list(range(NCORES))],
                ins=[partial[:].opt()],
                outs=[rsout[:].opt()],
            )
            fin = finp.tile([qr, D + 1], bf16, tag="fin", name=f"fin_{h}")
            nc.sync.dma_start(out=fin[:], in_=rsout[:])
            rz = finp.tile([qr, 1], f32, tag="rz", name=f"rz_{h}")
            nc.vector.reciprocal(rz[:], fin[:, D : D + 1])
            outb = finp.tile([qr, D], f32, tag="outb", name=f"outb_{h}")
            nc.vector.tensor_scalar_mul(outb[:], fin[:, 0:D], rz[:])
            nc.sync.dma_start(out=out_ext[roff : roff + qr, :], in_=outb[:])

        # ---- main: scores^T -> exp -> O (PSUM) / Z (DVE) accumulation ----
        # chunk h=0 is interleaved with the m-load loop (LAG tiles ahead)
        rs_jobs = []

        # pass-finalize: Z colsum + transpose-in, stage O|Z to bf16, DMA to
        # DRAM, ReduceScatter. For pass h < last this is EMITTED inside pass
        # h+1's pipeline so its PE ops (zsum, ztp) can't head-of-line-block
        # the next pass's matmul stream behind the DVE zacc drain.
        def _finalize(h, o2, zacc):
            qoff, qp = PASSES[h]
            qpt = qp // P
            zacc_bf = finp.tile([P, qp], bf16, tag="zbf", name=f"zbf_{h}")
            nc.vector.tensor_add(zacc_bf[:], zacc[:, 0, :], zacc[:, 1, :])
            zsum = psum_zt.tile([1, qp], f32, tag="tp", name=f"zsum_{h}")
            nc.tensor.matmul(
                zsum[:], ones_bf[:], zacc_bf[:], start=True, stop=True
            )
            zrow = finp.tile([1, qp], f32, tag="zrow", name=f"zrow_{h}")
            nc.vector.tensor_copy(out=zrow[:], in_=zsum[:])
            ztp = psum_zt.tile([P, qpt], f32, tag="tp", name=f"ztp_{h}")
            for j in range(qpt):
                nc.tensor.transpose(
                    ztp[:, j : j + 1], zrow[0:1, j * P : (j + 1) * P], one_f32[:]
                )
            # stage [128, qpt, D+1] bf16: cols 0..D-1 = O, col D = Z
            stage = stp.tile([P, qpt, D + 1], bf16, tag="stage", name=f"stage_{h}")
            for j in range(qpt):
                nc.vector.tensor_copy(out=stage[:, j, 0:D], in_=o2[j][:])
            nc.vector.tensor_copy(
                out=stage[:, :, D : D + 1],
                in_=ztp[:].rearrange("p (j o) -> p j o", o=1),
            )
            partial = dram.tile([qp, D + 1], bf16, tag="partial", name=f"partial_{h}")
            nc.sync.dma_start(
                out=partial[:].rearrange("(o p) d -> p o d", p=P),
                in_=stage[:],
            )
            if loop_cm is None:
                _emit_rs(h, partial)
            else:
                rs_jobs.append((h, partial))

        pending_fin = None
        for h, (qoff, qp) in enumerate(PASSES):
            qpt = qp // P
            o2 = []
            for j in range(qpt):
                o2.append(psum_o.tile([P, D], f32, tag="o2", name=f"o2_{h}_{j}"))
            # bf16 accumulator: 2x DVE throughput; partial sums stay ~O(100)
            # so bf16 rounding noise on Z is ~1e-3 relative -- negligible.
            zacc = zp.tile([P, 2, qp], bf16, tag="zacc", name=f"zacc_{h}")
            nc.gpsimd.memset(zacc[:], 0.0)

            def _mm1(t, h=h, qoff=qoff, qp=qp):
                sc = psum_sc.tile([P, qp], f32, tag="sc", name=f"sc_{h}_{t}")
                for c in range(DC):
                    nc.tensor.matmul(
                        sc[:],
                        mT[:, c, t * P : (t + 1) * P],
                        xhatT[:, c, qoff : qoff + qp],
                        start=(c == 0),
                        stop=(c == DC - 1),
                    )
                return sc

            # exp writes fp8 pt into pair-packed tiles [P, 2, qp]; one
            # DoubleRow matmul per (pair, q-chunk) contracts 256 s at once.
            ptp = {}

            def _exp(t, sc, h=h, ptp=ptp, zacc=zacc, qp=qp):
                u, i = t // 2, t % 2
                if i == 0:
                    ptp[u] = work.tile(
                        [P, 2, qp], fp8, tag="ptp", name=f"ptp_{h}_{u}"
                    )
                nc.scalar.activation(
                    out=ptp[u][:, i, :], in_=sc[:], func=AF.Exp,
                    scale=rs_m[:, t : t + 1],
                )
                if i == 1:
                    nc.vector.tensor_add(zacc[:], zacc[:], ptp[u][:])

            def _mm2(u, h=h, o2=o2, ptp=ptp, qpt=qpt):
                for j in range(qpt):
                    nc.tensor.matmul(
                        o2[j][:],
                        ptp[u][:, :, j * P : (j + 1) * P],
                        m_nat[:, 2 * u : 2 * u + 2, :],
                        start=(u == 0),
                        stop=(u == ST // 2 - 1),
                        perf_mode=mybir.MatmulPerfMode.DoubleRow,
                    )

            def _load(u):
                if h == 0:
                    load_tile_once(u)

            for u0 in range(LAG):
                _load(u0)
            sc0 = _mm1(0)
            _load(LAG)
            cur = _mm1(1)
            _exp(0, sc0)
            pend = (1, cur)
            for t in range(2, ST):
                _load(t + LAG - 1)
                sc = _mm1(t)
                if t == 3 and pending_fin is not None:
                    pending_fin()
                    pending_fin = None
                t1, sc_t1 = pend
                _exp(t1, sc_t1)
                if t1 % 2 == 1:
                    _mm2((t1 - 1) // 2)
                pend = (t, sc)
            t1, sc_t1 = pend
            _exp(t1, sc_t1)
            _mm2((t1 - 1) // 2)
            pending_fin = lambda h=h, o2=o2, zacc=zacc: _finalize(h, o2, zacc)
        pending_fin()
        pending_fin = None

        if loop_cm is not None:
            loop_cm.__exit__(None, None, None)
            for h, partial in rs_jobs:
                _emit_rs(h, partial)

    nc.compile()
    return nc


def _get_nc():
    if "nc" not in _CACHE:
        _CACHE["nc"] = _build()
    return _CACHE["nc"]


def _run(x, memory_bank, trace=False, **trace_kwargs):
    from concourse.bass_utils import run_bass_kernel_spmd

    nc = _get_nc()
    x = np.ascontiguousarray(np.asarray(x, dtype=np.float32))
    memory_bank = np.ascontiguousarray(np.asarray(memory_bank, dtype=np.float32))
    in_maps = [
        {
            "x": x,
            "mem": np.ascontiguousarray(
                memory_bank[i * S_SHARD : (i + 1) * S_SHARD]
            ),
        }
        for i in range(NCORES)
    ]
    res = run_bass_kernel_spmd(
        nc, in_maps, list(range(NCORES)), trace=trace, **trace_kwargs
    )
    # per pass (qoff, qp): core i's out rows [qoff//8 : qoff//8 + qp//8]
    # hold global q rows qoff + i*(qp//8) + k
    out = np.empty((B, D), dtype=np.float32)
    for i in range(NCORES):
        r = np.asarray(res.results[i]["out"])
        for qoff, qp in PASSES:
            qr = qp // NCORES
            out[qoff + i * qr : qoff + (i + 1) * qr] = r[
                qoff // NCORES : qoff // NCORES + qr
            ]
    return out, res


def kernel(x, memory_bank):
    out, _ = _run(x, memory_bank)
    return out


if __name__ == "__main__":
    xs = np.random.randn(B, D).astype(np.float32)
    ms = np.random.randn(S, D).astype(np.float32)
    o = kernel(xs, ms)
    print(o.shape, o.dtype)



# revision 38
# speedup vs baseline: 1.0084x; 1.0084x over previous
"""Trainium2 Bass kernel for nn_MemorySystem (cosine-sim attention memory read).

reference:
    x_norm = ||x||_row (B,1); m_norm = ||m||_row (S,1)
    sims = (x @ m^T) / max(x_norm * m_norm^T, 1e-8)
    attn = softmax(8.0 * sims, axis=1)
    out  = attn @ m                       # (B, D)

Sharding: memory_bank rows split across 8 NeuronCores (8192 rows each).
Each core computes, for its shard, the un-normalized softmax numerator
O_c = exp(S_c) @ m_c (B, D) and denominator Z_c = sum_s exp (B,), using the
bounded-score property (|8*cos| <= 8) to skip the max-subtraction entirely.
Per 512-query pass, a bf16 ReduceScatter(add) over the [512, 513] partials
(O|Z) leaves each core with a fully-reduced 64-query slice; it divides O/Z
on-chip and emits those rows. The host reassembles the slices. Pass h's
finalize chain (Z colsum, staging, RS) is emitted inside pass h+1's pipeline
so its PE ops can't head-of-line-block the next pass's matmul stream; only
the last pass's ReduceScatter is exposed at the tail.

On-chip structure (per core):
  - x is normalized on load (1/||x_q|| folded in) and transposed via TensorE
    to xhatT [d, q] (bf16).
  - m shard is SBUF-resident in TWO layouts: transposed [d, s] bf16 (lhsT of
    the 1st matmul) and natural [s, d] fp8-e4m3 (rhs of the 2nd matmul).
  - scores are computed transposed, [s-tile, q], so 8/||m_s|| is a
    per-partition scalar fused into the ACT Exp; exp writes fp8-e4m3 directly
    (values in ~[0.1, 11] are safely inside e4m3 range), pair-packed so two
    adjacent s-tiles feed ONE DoubleRow fp8 matmul (contraction 256) in the
    2nd matmul -- ~1.8x fewer PE cycles there vs bf16. Z is accumulated from
    the SAME quantized fp8 pt, so the softmax renormalization cancels the
    common quantization bias (measured end-to-end rel err 7.4e-3 in numpy).
  - row norms: ACT Square (in every ACT table set -> no table thrash with
    Exp) with accum_out and scale=1/8 giving n2/64; rsqrt via DVE
    bit-trick + 2 Newton steps (no ACT Sqrt -> no table swaps).
  - m load chain: one DVE f32->bf16 cast feeds ACT Square, cheap bf16 PE
    transposes, the fp8 m_nat recast and a 16-bit PSUM copyback.
  - Z is accumulated on DVE in bf16 (pair-wide adds of the fp8 exp tiles),
    cross-partition-reduced by one ones-matmul per pass; the division
    happens after the ReduceScatter.
  - pass 0 of the query loop is emission-interleaved with the m-load loop
    so the in-order PE stream overlaps DMA/cast/norm work with matmuls.

Measured on 8 axon trn2 cores: ~297-311 us HW exec (baseline 355 us),
rel err 8.9e-3 (gate 2e-2).
"""

import sys

sys.path.insert(0, "/opt/trn_rl_repo")

import numpy as np
from contextlib import ExitStack

B, S, D = 1024, 65536, 512
NCORES = 8
S_SHARD = S // NCORES  # 8192
P = 128

ST = S_SHARD // P  # 64 s-tiles per core
QT = B // P  # 8 q-tiles
DC = D // P  # 4 d-chunks
# query passes (offset, size): each pass's ReduceScatter overlaps the next
# pass's compute; only the last one is exposed.
PASSES = [(0, 512), (512, 512)]
LAG = 5  # load runs this many s-tiles ahead of chunk-0 compute

MAGIC = 0x5F3759DF

_CACHE = {}


def _build(loop_iters=None):
    """Build the kernel. loop_iters wraps the whole body in a device-side
    repeat loop (used only for wall-clock delta timing in bench.py)."""
    import concourse.bass as bass
    import concourse.tile as tile
    from concourse import bacc, mybir
    from concourse.masks import make_identity

    f32 = mybir.dt.float32
    bf16 = mybir.dt.bfloat16
    fp8 = mybir.dt.float8e4
    u32 = mybir.dt.uint32
    AF = mybir.ActivationFunctionType
    ALU = mybir.AluOpType

    nc = bacc.Bacc(None, num_devices=NCORES)
    x_ext = nc.declare_dram_parameter("x", [B, D], f32, isOutput=False)
    m_ext = nc.declare_dram_parameter("mem", [S_SHARD, D], f32, isOutput=False)
    out_ext = nc.declare_dram_parameter("out", [B // NCORES, D], f32, isOutput=True)

    with tile.TileContext(nc) as tc, ExitStack() as ctx:
        persist = ctx.enter_context(tc.tile_pool(name="persist", bufs=1))
        loadp = ctx.enter_context(tc.tile_pool(name="load", bufs=6))
        xp = ctx.enter_context(tc.tile_pool(name="xp", bufs=QT))
        sqp = ctx.enter_context(tc.tile_pool(name="sqp", bufs=2))
        work = ctx.enter_context(tc.tile_pool(name="work", bufs=3))
        zp = ctx.enter_context(tc.tile_pool(name="zp", bufs=2))
        stp = ctx.enter_context(tc.tile_pool(name="stp", bufs=2))
        finp = ctx.enter_context(tc.tile_pool(name="finp", bufs=1))
        dram = ctx.enter_context(tc.tile_pool(name="dram", bufs=4, space="DRAM"))
        # PSUM: 8 banks total. sc(2) + o2(4) + tp(2, shared with zt) = 8
        psum_sc = ctx.enter_context(tc.tile_pool(name="psc", bufs=2, space="PSUM"))
        psum_o = ctx.enter_context(tc.tile_pool(name="po", bufs=4, space="PSUM"))
        psum_tp = ctx.enter_context(tc.tile_pool(name="ptp", bufs=2, space="PSUM"))
        psum_zt = psum_tp

        # ---- constants ----
        ident_bf = persist.tile([P, P], bf16)
        make_identity(nc, ident_bf[:])
        ident_f32 = persist.tile([P, P], f32)
        nc.vector.tensor_copy(out=ident_f32[:], in_=ident_bf[:])
        ones_f32 = persist.tile([P, 1], f32)
        nc.vector.memset(ones_f32[:], 1.0)
        ones_bf = persist.tile([P, 1], bf16)
        nc.vector.memset(ones_bf[:], 1.0)
        one_f32 = persist.tile([1, 1], f32)
        nc.vector.memset(one_f32[:], 1.0)
        magic_u = persist.tile([P, 1], u32)
        nc.vector.memset(magic_u[:], MAGIC)

        loop_cm = tc.For_i(0, loop_iters, 1) if loop_iters else None
        if loop_cm is not None:
            loop_cm.__enter__()

        # ---- persistent SBUF tensors ----
        m_nat = persist.tile([P, ST, D], fp8)  # [s%128, s//128, d] (mm2 rhs)
        mT = persist.tile([P, DC, S_SHARD], bf16)  # [d%128, d//128, s]
        xhatT = persist.tile([P, DC, B], bf16)  # [d%128, d//128, q]
        n2m = persist.tile([P, ST], f32)  # ||m_s||^2 / 64
        rs_m = persist.tile([P, ST], f32)  # 8 / ||m_s||
        rs_u = persist.tile([P, ST], u32)  # newton scratch (bit-trick y)
        rs_t = persist.tile([P, ST], f32)  # newton scratch t1
        xn2 = persist.tile([P, QT], f32)
        rs_x = persist.tile([P, QT], f32)
        xr_u = persist.tile([P, QT], u32)
        xr_t = persist.tile([P, QT], f32)

        def rsqrt_newton(dst, a, uscr, tscr, n):
            """dst = 1/sqrt(a); all APs [P, n] f32 (uscr u32)."""
            mg = magic_u[:, 0:1]
            if n > 1:
                mg = mg.to_broadcast((P, n))
            nc.vector.tensor_scalar(
                uscr, a.bitcast(u32), 1, None, ALU.logical_shift_right
            )
            nc.vector.tensor_tensor(uscr, mg, uscr, ALU.subtract)
            y = uscr.bitcast(f32)
            for it in range(2):
                out_y = dst if it == 1 else y
                nc.vector.tensor_tensor(tscr, y, y, ALU.mult)
                nc.vector.tensor_tensor(tscr, tscr, a, ALU.mult)
                nc.vector.tensor_scalar(tscr, tscr, -0.5, 1.5, ALU.mult, ALU.add)
                nc.vector.tensor_tensor(out_y, y, tscr, ALU.mult)

        # ---- per-s-tile load step (DMA, cast, norms, transpose) ----
        # One DVE cast to bf16 feeds everything downstream: ACT Square for
        # norms, cheap bf16 PE transposes, a 16-bit fp8 recast for m_nat,
        # and a 16-bit PSUM copyback -- minimizes both PE transpose cycles
        # and total DVE bytes.
        def load_tile(t):
            mf = loadp.tile([P, D], f32, tag="mf32", name=f"mf_{t}")
            nc.sync.dma_start(out=mf[:], in_=m_ext[t * P : (t + 1) * P, :])
            mb = work.tile([P, D], bf16, tag="mb16", name=f"mb_{t}")
            nc.vector.tensor_copy(out=mb[:], in_=mf[:])
            msq = sqp.tile([P, D], f32, tag="sq", name=f"msq_{t}")
            # scale=1/8: accum collects sum((m/8)^2) = n2/64; rsqrt -> 8/||m||
            nc.scalar.activation(
                out=msq[:], in_=mb[:], func=AF.Square, scale=0.125,
                accum_out=n2m[:, t : t + 1],
            )
            if t % 4 == 3:
                s = slice(t - 3, t + 1)
                rsqrt_newton(rs_m[:, s], n2m[:, s], rs_u[:, s], rs_t[:, s], 4)
            nc.vector.tensor_copy(out=m_nat[:, t, :], in_=mb[:])
            mtp = psum_tp.tile([P, DC * P], bf16, tag="tp", name=f"mtp_{t}")
            for c in range(DC):
                nc.tensor.transpose(
                    mtp[:, c * P : (c + 1) * P],
                    mb[:, c * P : (c + 1) * P],
                    ident_bf[:],
                )
            nc.vector.tensor_copy(
                out=mT[:, :, t * P : (t + 1) * P],
                in_=mtp[:].rearrange("p (c q) -> p c q", c=DC),
            )

        loaded = set()

        def load_tile_once(t):
            if t < ST and t not in loaded:
                loaded.add(t)
                load_tile(t)

        # prime the m pipeline before x-prep so DMA/ACT start immediately
        def prime():
            for u in range(LAG):
                load_tile_once(u)

        prime()

        # ---- x prep (single DMA; xf tiles stay resident for pass 2) ----
        xfs = []
        for j in range(QT):
            xf = xp.tile([P, D], f32, tag="xf", name=f"xf_{j}")
            xfs.append(xf)
            nc.sync.dma_start(out=xf[:], in_=x_ext[j * P : (j + 1) * P, :])
            xsq = sqp.tile([P, D], f32, tag="sq", name=f"xsq_{j}")
            nc.scalar.activation(
                out=xsq[:], in_=xf[:], func=AF.Square,
                accum_out=xn2[:, j : j + 1],
            )
        rsqrt_newton(rs_x[:], xn2[:], xr_u[:], xr_t[:], QT)
        for j in range(QT):
            xhat = work.tile([P, D], bf16, tag="xhat", name=f"xhat_{j}")
            nc.vector.tensor_scalar_mul(xhat[:], xfs[j][:], rs_x[:, j : j + 1])
            xtp = psum_tp.tile([P, DC * P], bf16, tag="tp", name=f"xtp_{j}")
            for c in range(DC):
                nc.tensor.transpose(
                    xtp[:, c * P : (c + 1) * P],
                    xhat[:, c * P : (c + 1) * P],
                    ident_bf[:],
                )
            nc.vector.tensor_copy(
                out=xhatT[:, :, j * P : (j + 1) * P],
                in_=xtp[:].rearrange("p (c q) -> p c q", c=DC),
            )

        # per-chunk ReduceScatter + divide + output (bf16 on the wire: the
        # CCE adds in fp32, payload quantization adds <1e-4 to rel err)
        def _emit_rs(h, partial):
            qoff, qp = PASSES[h]
            qr = qp // NCORES
            roff = qoff // NCORES
            rsout = dram.tile([qr, D + 1], bf16, tag="rsout", name=f"rsout_{h}")
            nc.gpsimd.collective_compute(
                "ReduceScatter",
                mybir.AluOpType.add,
                replica_groups=[list(range(NCORES))],
                ins=[partial[:].opt()],
                outs=[rsout[:].opt()],
            )
            fin = finp.tile([qr, D + 1], bf16, tag="fin", name=f"fin_{h}")
            nc.sync.dma_start(out=fin[:], in_=rsout[:])
            rz = finp.tile([qr, 1], f32, tag="rz", name=f"rz_{h}")
            nc.vector.reciprocal(rz[:], fin[:, D : D + 1])
            outb = finp.tile([qr, D], f32, tag="outb", name=f"outb_{h}")
            nc.vector.tensor_scalar_mul(outb[:], fin[:, 0:D], rz[:])
            nc.sync.dma_start(out=out_ext[roff : roff + qr, :], in_=outb[:])

        # ---- main: scores^T -> exp -> O (PSUM) / Z (DVE) accumulation ----
        # chunk h=0 is interleaved with the m-load loop (LAG tiles ahead)
        rs_jobs = []

        # pass-finalize: Z colsum + transpose-in, stage O|Z to bf16, DMA to
        # DRAM, ReduceScatter. For pass h < last this is EMITTED inside pass
        # h+1's pipeline so its PE ops (zsum, ztp) can't head-of-line-block
        # the next pass's matmul stream behind the DVE zacc drain.
        def _finalize(h, o2, zacc):
            qoff, qp = PASSES[h]
            qpt = qp // P
            zacc_bf = finp.tile([P, qp], bf16, tag="zbf", name=f"zbf_{h}")
            nc.vector.tensor_add(zacc_bf[:], zacc[:, 0, :], zacc[:, 1, :])
            zsum = psum_zt.tile([1, qp], f32, tag="tp", name=f"zsum_{h}")
            nc.tensor.matmul(
                zsum[:], ones_bf[:], zacc_bf[:], start=True, stop=True
            )
            zrow = finp.tile([1, qp], f32, tag="zrow", name=f"zrow_{h}")
            nc.vector.tensor_copy(out=zrow[:], in_=zsum[:])
            ztp = psum_zt.tile([P, qpt], f32, tag="tp", name=f"ztp_{h}")
            for j in range(qpt):
                nc.tensor.transpose(
                    ztp[:, j : j + 1], zrow[0:1, j * P : (j + 1) * P], one_f32[:]
                )
            # stage [128, qpt, D+1] bf16: cols 0..D-1 = O, col D = Z
            stage = stp.tile([P, qpt, D + 1], bf16, tag="stage", name=f"stage_{h}")
            for j in range(qpt):
                nc.vector.tensor_copy(out=stage[:, j, 0:D], in_=o2[j][:])
            nc.vector.tensor_copy(
                out=stage[:, :, D : D + 1],
                in_=ztp[:].rearrange("p (j o) -> p j o", o=1),
            )
            partial = dram.tile([qp, D + 1], bf16, tag="partial", name=f"partial_{h}")
            nc.sync.dma_start(
                out=partial[:].rearrange("(o p) d -> p o d", p=P),
                in_=stage[:],
            )
            if loop_cm is None:
                _emit_rs(h, partial)
            else:
                rs_jobs.append((h, partial))

        pending_fin = None
        for h, (qoff, qp) in enumerate(PASSES):
            qpt = qp // P
            o2 = []
            for j in range(qpt):
                o2.append(psum_o.tile([P, D], f32, tag="o2", name=f"o2_{h}_{j}"))
            # bf16 accumulator: 2x DVE throughput; partial sums stay ~O(100)
            # so bf16 rounding noise on Z is ~1e-3 relative -- negligible.
            zacc = zp.tile([P, 2, qp], bf16, tag="zacc", name=f"zacc_{h}")
            nc.gpsimd.memset(zacc[:], 0.0)

            def _mm1(t, h=h, qoff=qoff, qp=qp):
                sc = psum_sc.tile([P, qp], f32, tag="sc", name=f"sc_{h}_{t}")
                for c in range(DC):
                    nc.tensor.matmul(
                        sc[:],
                        mT[:, c, t * P : (t + 1) * P],
                        xhatT[:, c, qoff : qoff + qp],
                        start=(c == 0),
                        stop=(c == DC - 1),
                    )
                return sc

            # exp writes fp8 pt into pair-packed tiles [P, 2, qp]; one
            # DoubleRow matmul per (pair, q-chunk) contracts 256 s at once.
            ptp = {}

            def _exp(t, sc, h=h, ptp=ptp, zacc=zacc, qp=qp):
                u, i = t // 2, t % 2
                if i == 0:
                    ptp[u] = work.tile(
                        [P, 2, qp], fp8, tag="ptp", name=f"ptp_{h}_{u}"
                    )
                nc.scalar.activation(
                    out=ptp[u][:, i, :], in_=sc[:], func=AF.Exp,
                    scale=rs_m[:, t : t + 1],
                )
                if i == 1:
                    nc.vector.tensor_add(zacc[:], zacc[:], ptp[u][:])

            def _mm2(u, h=h, o2=o2, ptp=ptp, qpt=qpt):
                for j in range(qpt):
                    nc.tensor.matmul(
                        o2[j][:],
                        ptp[u][:, :, j * P : (j + 1) * P],
                        m_nat[:, 2 * u : 2 * u + 2, :],
                        start=(u == 0),
                        stop=(u == ST // 2 - 1),
                        perf_mode=mybir.MatmulPerfMode.DoubleRow,
                    )

            def _load(u):
                if h == 0:
                    load_tile_once(u)

            for u0 in range(LAG):
                _load(u0)
            sc0 = _mm1(0)
            _load(LAG)
            cur = _mm1(1)
            _exp(0, sc0)
            pend = (1, cur)
            for t in range(2, ST):
                _load(t + LAG - 1)
                sc = _mm1(t)
                if t == 3 and pending_fin is not None:
                    pending_fin()
                    pending_fin = None
                t1, sc_t1 = pend
                _exp(t1, sc_t1)
                if t1 % 2 == 1:
                    _mm2((t1 - 1) // 2)
                pend = (t, sc)
            t1, sc_t1 = pend
            _exp(t1, sc_t1)
            _mm2((t1 - 1) // 2)
            pending_fin = lambda h=h, o2=o2, zacc=zacc: _finalize(h, o2, zacc)
        pending_fin()
        pending_fin = None

        if loop_cm is not None:
            loop_cm.__exit__(None, None, None)
            for h, partial in rs_jobs:
                _emit_rs(h, partial)

    nc.compile()
    return nc


def _get_nc():
    if "nc" not in _CACHE:
        _CACHE["nc"] = _build()
    return _CACHE["nc"]


def _run(x, memory_bank, trace=False, **trace_kwargs):
    from concourse.bass_utils import run_bass_kernel_spmd

    nc = _get_nc()
    x = np.ascontiguousarray(np.asarray(x, dtype=np.float32))
    memory_bank = np.ascontiguousarray(np.asarray(memory_bank, dtype=np.float32))
    in_maps = [
        {
            "x": x,
            "mem": np.ascontiguousarray(
                memory_bank[i * S_SHARD : (i + 1) * S_SHARD]
            ),
        }
        for i in range(NCORES)
    ]
    res = run_bass_kernel_spmd(
        nc, in_maps, list(range(NCORES)), trace=trace, **trace_kwargs
    )
    # per pass (qoff, qp): core i's out rows [qoff//8 : qoff//8 + qp//8]
    # hold global q rows qoff + i*(qp//8) + k
    out = np.empty((B, D), dtype=np.float32)
    for i in range(NCORES):
        r = np.asarray(res.results[i]["out"])
        for qoff, qp in PASSES:
            qr = qp // NCORES
            out[qoff + i * qr : qoff + (i + 1) * qr] = r[
                qoff // NCORES : qoff // NCORES + qr
            ]
    return out, res


def kernel(x, memory_bank):
    out, _ = _run(x, memory_bank)
    return out


if __name__ == "__main__":
    xs = np.random.randn(B, D).astype(np.float32)
    ms = np.random.randn(S, D).astype(np.float32)
    o = kernel(xs, ms)
    print(o.shape, o.dtype)

